# revision 10
# baseline (speedup 1.0000x reference)
# Trainium2 Bass kernel for nn_MultiCondLayer:
#   out[b,o,n] = (sum_k (cond[b] @ W[k].T)[o,n] + sum_k b[k,o]) * x_mask[b,0,n]
# Key algebraic reduction: sum_k Linear_k(x) == Linear(x) with W' = sum_k W[k],
# b' = sum_k b[k]  (4x FLOP reduction vs. the naive einsum over k).
#
# Sharding: data-parallel over batch B=8 across the 8 NeuronCores (one batch
# element per core); the reduced [1024,1024] weight is replicated.
# Per-core compute: [1024c,4096n] activations x [1024c,1024o] weights as
# 512 PE matmuls (128x128 lhsT, 128x512 rhs, fp32r) accumulating in PSUM,
# evicted by a single fused DVE op: (psum + bias) * mask.

import numpy as np
from contextlib import ExitStack

import concourse.bass as bass
import concourse.mybir as mybir
import concourse.tile as tile
from concourse import bacc
from concourse.bass_utils import run_bass_kernel_spmd

P = 128
B, C, N = 8, 1024, 4096
O = 1024
NT = 512                 # matmul free dim = one fp32 PSUM bank
CO, OO, NN = C // P, O // P, N // NT
F32 = mybir.dt.float32
F32R = mybir.dt.float32r

N_CORES = 8


NSUP = 1024              # n superchunk width (4 KB DMA descriptors)
NSUPS = N // NSUP        # 4
NSUB = NSUP // NT        # 2 psum-width subchunks per superchunk


def build_module():
    nc = bacc.Bacc("TRN2", target_bir_lowering=False, debug=False,
                   num_devices=N_CORES)
    x = nc.dram_tensor("x", [C, N], F32R, kind="ExternalInput")    # cond[b]
    wt = nc.dram_tensor("wt", [C, O], F32R, kind="ExternalInput")  # (sum_k W[k]).T
    bv = nc.dram_tensor("bv", [O], F32, kind="ExternalInput")      # sum_k b[k]
    mk = nc.dram_tensor("mk", [N], F32R, kind="ExternalInput")      # x_mask[b,0]
    out = nc.dram_tensor("out", [O, N], F32, kind="ExternalOutput")

    x_r = x.ap().rearrange("(c p) n -> p c n", p=P)      # [128, CO, N]
    wt_r = wt.ap().rearrange("(c p) o -> p c o", p=P)    # [128, CO, O]
    bv_r = bv.ap().rearrange("(j p) -> p j", p=P)        # [128, OO]

    # DMA queue split: x-in alone on the Sync HWDGE queue; consts (bias ->
    # per-c weights -> mask) then out-stores on the Activation HWDGE queue.
    # Mask goes AFTER the weights: first needed by the first eviction
    # (~15us in), while the weights gate the matmul stream from ~2us.
    with tile.TileContext(nc) as tc:
        with (
            tc.tile_pool(name="consts", bufs=1) as consts,
            tc.tile_pool(name="xs", bufs=2) as xs,
            tc.tile_pool(name="outs", bufs=12) as outs,
            tc.tile_pool(name="ps", bufs=8, space="PSUM") as psp,
        ):
            # Mask broadcast: one partition-broadcast DMA (step-0 partition
            # dim) alone on the otherwise-idle GpSimd SWDGE queue; lands well
            # before the first eviction needs it. GpSimd DMA may cast
            # f32r -> f32.
            mask_sb = consts.tile([P, N], F32)
            nc.gpsimd.dma_start(mask_sb[:], mk.ap()[None, :].broadcast_to([P, N]))
            bias_sb = consts.tile([P, OO], F32)
            nc.scalar.dma_start(bias_sb[:], bv_r)
            w_sb = consts.tile([P, CO, O], F32R)
            for c in range(CO):
                nc.scalar.dma_start(w_sb[:, c, :], wt_r[:, c, :])

            for ns in range(NSUPS):
                x_sb = xs.tile([P, CO, NSUP], F32R)
                for c in range(CO):
                    nc.sync.dma_start(
                        x_sb[:, c, :], x_r[:, c, ns * NSUP:(ns + 1) * NSUP])
                for nsub in range(NSUB):
                    nlo = nsub * NT
                    n0 = ns * NSUP + nlo
                    pss = [psp.tile([P, NT], F32, name=f"ps_{ns}_{nsub}_{o}", tag="ps")
                           for o in range(OO)]
                    for c in range(CO):
                        for o in range(OO):
                            nc.tensor.matmul(
                                pss[o][:],
                                w_sb[:, c, o * P:(o + 1) * P],
                                x_sb[:, c, nlo:nlo + NT],
                                start=(c == 0),
                                stop=(c == CO - 1),
                            )
                    for o in range(OO):
                        ot = outs.tile([P, NT], F32, name=f"ot_{ns}_{nsub}_{o}",
                                       tag="ot")
                        nc.vector.scalar_tensor_tensor(
                            ot[:], pss[o][:],
                            bias_sb[:, o:o + 1], mask_sb[:, n0:n0 + NT],
                            op0=mybir.AluOpType.add, op1=mybir.AluOpType.mult,
                        )
                        nc.scalar.dma_start(
                            out.ap()[o * P:(o + 1) * P, n0:n0 + NT], ot[:])
    nc.compile()
    return nc


_NC_CACHE = None


def _get_module():
    global _NC_CACHE
    if _NC_CACHE is None:
        _NC_CACHE = build_module()
    return _NC_CACHE


def _make_in_maps(cond, x_mask, W, b):
    wt = np.ascontiguousarray(W.sum(axis=0).T, dtype=np.float32)   # [C, O]
    bv = np.ascontiguousarray(b.sum(axis=0), dtype=np.float32)     # [O]
    in_maps = []
    for core in range(N_CORES):
        in_maps.append({
            "x": np.ascontiguousarray(cond[core], dtype=np.float32),
            "wt": wt,
            "bv": bv,
            "mk": np.ascontiguousarray(x_mask[core, 0], dtype=np.float32),
        })
    return in_maps


def run(cond, x_mask, W, b, trace=False, trace_cores=None):
    """Run on hardware; returns (out [B,O,N] fp32, BassKernelResults)."""
    nc = _get_module()
    in_maps = _make_in_maps(cond, x_mask, W, b)
    res = run_bass_kernel_spmd(
        nc, in_maps, core_ids=list(range(N_CORES)),
        trace=trace, trace_cores=trace_cores,
    )
    out = np.stack([res.results[i]["out"] for i in range(N_CORES)], axis=0)
    return out, res


def kernel(cond, x_mask, W, b):
    out, _ = run(cond, x_mask, W, b)
    return out


# revision 12
# speedup vs baseline: 1.1267x; 1.1267x over previous
# Trainium2 Bass kernel for nn_MultiCondLayer:
#   out[b,o,n] = (sum_k (cond[b] @ W[k].T)[o,n] + sum_k b[k,o]) * x_mask[b,0,n]
# Key algebraic reduction: sum_k Linear_k(x) == Linear(x) with W' = sum_k W[k],
# b' = sum_k b[k]  (4x FLOP reduction vs. the naive einsum over k).
#
# Sharding: data-parallel over batch B=8 across the 8 NeuronCores (one batch
# element per core); the reduced [1024,1024] weight is replicated.
# Per-core compute: [1024c,4096n] activations x [1024c,1024o] weights as
# 512 PE matmuls (128x128 lhsT, 128x512 rhs, fp32r) accumulating in PSUM,
# evicted by a single fused DVE op: (psum + bias) * mask.

import numpy as np
from contextlib import ExitStack

import concourse.bass as bass
import concourse.mybir as mybir
import concourse.tile as tile
from concourse import bacc
from concourse.bass_utils import run_bass_kernel_spmd

P = 128
B, C, N = 8, 1024, 4096
O = 1024
NT = 512                 # matmul free dim = one fp32 PSUM bank
CO, OO, NN = C // P, O // P, N // NT
F32 = mybir.dt.float32
F32R = mybir.dt.float32r

N_CORES = 8


NSUP = 1024              # n superchunk width (4 KB DMA descriptors)
NSUPS = N // NSUP        # 4
NSUB = NSUP // NT        # 2 psum-width subchunks per superchunk


def build_module():
    nc = bacc.Bacc("TRN2", target_bir_lowering=False, debug=False,
                   num_devices=N_CORES)
    x = nc.dram_tensor("x", [C, N], F32R, kind="ExternalInput")    # cond[b]
    wt = nc.dram_tensor("wt", [C, O], F32R, kind="ExternalInput")  # (sum_k W[k]).T
    bv = nc.dram_tensor("bv", [O], F32, kind="ExternalInput")      # sum_k b[k]
    mk = nc.dram_tensor("mk", [N], F32R, kind="ExternalInput")      # x_mask[b,0]
    out = nc.dram_tensor("out", [O, N], F32, kind="ExternalOutput")

    x_r = x.ap().rearrange("(c p) n -> p c n", p=P)      # [128, CO, N]
    wt_r = wt.ap().rearrange("(c p) o -> p c o", p=P)    # [128, CO, O]
    bv_r = bv.ap().rearrange("(j p) -> p j", p=P)        # [128, OO]

    # DMA queue split: x-in alone on the Sync HWDGE queue; consts (bias ->
    # per-c weights -> mask) then out-stores on the Activation HWDGE queue.
    # Mask goes AFTER the weights: first needed by the first eviction
    # (~15us in), while the weights gate the matmul stream from ~2us.
    with tile.TileContext(nc) as tc:
        with (
            tc.tile_pool(name="consts", bufs=1) as consts,
            tc.tile_pool(name="xs", bufs=2) as xs,
            tc.tile_pool(name="outs", bufs=12) as outs,
            tc.tile_pool(name="ps", bufs=6, space="PSUM") as psp,
        ):
            # Mask broadcast built on-chip: the 16 KB mask row lands
            # instantly, then the (idle, cold) PE outer-products it with a
            # ones column into all 128 partitions, through a DEDICATED 2-bank
            # PSUM tag so the mask matmuls never contend with the main
            # stream's PSUM slots. Avoids a 2 MiB HBM DMA in the congested
            # startup window entirely.
            bias_sb = consts.tile([P, OO], F32)
            nc.scalar.dma_start(bias_sb[:], bv_r)
            mkrow_sb = consts.tile([1, N], F32R)
            nc.scalar.dma_start(mkrow_sb[:], mk.ap()[None, :])
            ones_sb = consts.tile([1, P], F32)
            nc.vector.memset(ones_sb[:], 1.0)
            mask_sb = consts.tile([P, N], F32)
            for n in range(NN):
                mps = psp.tile([P, NT], F32, name=f"mps_{n}", tag="mps", bufs=2)
                nc.tensor.matmul(mps[:], ones_sb[:].bitcast(F32R),
                                 mkrow_sb[:, n * NT:(n + 1) * NT],
                                 start=True, stop=True)
                nc.vector.tensor_copy(mask_sb[:, n * NT:(n + 1) * NT], mps[:])
            w_sb = consts.tile([P, CO, O], F32R)
            for c in range(CO):
                nc.scalar.dma_start(w_sb[:, c, :], wt_r[:, c, :])

            for ns in range(NSUPS):
                x_sb = xs.tile([P, CO, NSUP], F32R)
                for c in range(CO):
                    nc.sync.dma_start(
                        x_sb[:, c, :], x_r[:, c, ns * NSUP:(ns + 1) * NSUP])
                for nsub in range(NSUB):
                    nlo = nsub * NT
                    n0 = ns * NSUP + nlo
                    pss = [psp.tile([P, NT], F32, name=f"ps_{ns}_{nsub}_{o}", tag="ps", bufs=6)
                           for o in range(OO)]
                    for c in range(CO):
                        for o in range(OO):
                            nc.tensor.matmul(
                                pss[o][:],
                                w_sb[:, c, o * P:(o + 1) * P],
                                x_sb[:, c, nlo:nlo + NT],
                                start=(c == 0),
                                stop=(c == CO - 1),
                            )
                    for o in range(OO):
                        ot = outs.tile([P, NT], F32, name=f"ot_{ns}_{nsub}_{o}",
                                       tag="ot")
                        nc.vector.scalar_tensor_tensor(
                            ot[:], pss[o][:],
                            bias_sb[:, o:o + 1], mask_sb[:, n0:n0 + NT],
                            op0=mybir.AluOpType.add, op1=mybir.AluOpType.mult,
                        )
                        nc.scalar.dma_start(
                            out.ap()[o * P:(o + 1) * P, n0:n0 + NT], ot[:])
    nc.compile()
    return nc


_NC_CACHE = None


def _get_module():
    global _NC_CACHE
    if _NC_CACHE is None:
        _NC_CACHE = build_module()
    return _NC_CACHE


def _make_in_maps(cond, x_mask, W, b):
    wt = np.ascontiguousarray(W.sum(axis=0).T, dtype=np.float32)   # [C, O]
    bv = np.ascontiguousarray(b.sum(axis=0), dtype=np.float32)     # [O]
    in_maps = []
    for core in range(N_CORES):
        in_maps.append({
            "x": np.ascontiguousarray(cond[core], dtype=np.float32),
            "wt": wt,
            "bv": bv,
            "mk": np.ascontiguousarray(x_mask[core, 0], dtype=np.float32),
        })
    return in_maps


def run(cond, x_mask, W, b, trace=False, trace_cores=None):
    """Run on hardware; returns (out [B,O,N] fp32, BassKernelResults)."""
    nc = _get_module()
    in_maps = _make_in_maps(cond, x_mask, W, b)
    res = run_bass_kernel_spmd(
        nc, in_maps, core_ids=list(range(N_CORES)),
        trace=trace, trace_cores=trace_cores,
    )
    out = np.stack([res.results[i]["out"] for i in range(N_CORES)], axis=0)
    return out, res


def kernel(cond, x_mask, W, b):
    out, _ = run(cond, x_mask, W, b)
    return out


# revision 14
# speedup vs baseline: 1.1757x; 1.0435x over previous
# Trainium2 Bass kernel for nn_MultiCondLayer:
#   out[b,o,n] = (sum_k (cond[b] @ W[k].T)[o,n] + sum_k b[k,o]) * x_mask[b,0,n]
# Key algebraic reduction: sum_k Linear_k(x) == Linear(x) with W' = sum_k W[k],
# b' = sum_k b[k]  (4x FLOP reduction vs. the naive einsum over k).
#
# Sharding: data-parallel over batch B=8 across the 8 NeuronCores (one batch
# element per core); the reduced [1024,1024] weight is replicated.
# Per-core compute: [1024c,4096n] activations x [1024c,1024o] weights as
# 512 PE matmuls (128x128 lhsT, 128x512 rhs, fp32r) accumulating in PSUM,
# evicted by a single fused DVE op: (psum + bias) * mask.

import numpy as np
from contextlib import ExitStack

import concourse.bass as bass
import concourse.mybir as mybir
import concourse.tile as tile
from concourse import bacc
from concourse.bass_utils import run_bass_kernel_spmd

P = 128
B, C, N = 8, 1024, 4096
O = 1024
NT = 512                 # matmul free dim = one fp32 PSUM bank
CO, OO, NN = C // P, O // P, N // NT
F32 = mybir.dt.float32
F32R = mybir.dt.float32r

N_CORES = 8


NSUP = 1024              # n superchunk width (4 KB DMA descriptors)
NSUPS = N // NSUP        # 4
NSUB = NSUP // NT        # 2 psum-width subchunks per superchunk


def build_module():
    nc = bacc.Bacc("TRN2", target_bir_lowering=False, debug=False,
                   num_devices=N_CORES)
    x = nc.dram_tensor("x", [C, N], F32R, kind="ExternalInput")    # cond[b]
    wt = nc.dram_tensor("wt", [C, O], F32R, kind="ExternalInput")  # (sum_k W[k]).T
    bv = nc.dram_tensor("bv", [O], F32, kind="ExternalInput")      # sum_k b[k]
    mk = nc.dram_tensor("mk", [N], F32R, kind="ExternalInput")      # x_mask[b,0]
    out = nc.dram_tensor("out", [O, N], F32, kind="ExternalOutput")

    x_r = x.ap().rearrange("(c p) n -> p c n", p=P)      # [128, CO, N]
    wt_r = wt.ap().rearrange("(c p) o -> p c o", p=P)    # [128, CO, O]
    bv_r = bv.ap().rearrange("(j p) -> p j", p=P)        # [128, OO]

    # DMA queue split: x-in alone on the Sync HWDGE queue; consts (bias ->
    # per-c weights -> mask) then out-stores on the Activation HWDGE queue.
    # Mask goes AFTER the weights: first needed by the first eviction
    # (~15us in), while the weights gate the matmul stream from ~2us.
    with tile.TileContext(nc) as tc:
        with (
            tc.tile_pool(name="consts", bufs=1) as consts,
            tc.tile_pool(name="xs", bufs=2) as xs,
            tc.tile_pool(name="outs", bufs=16) as outs,
            tc.tile_pool(name="ps", bufs=8, space="PSUM") as psp,
        ):
            # Mask broadcast built on-chip: the 16 KB mask row lands
            # instantly, then the (idle, cold) PE outer-products it with a
            # ones column into all 128 partitions, through a DEDICATED 2-bank
            # PSUM tag so the mask matmuls never contend with the main
            # stream's PSUM slots. Avoids a 2 MiB HBM DMA in the congested
            # startup window entirely.
            bias_sb = consts.tile([P, OO], F32)
            nc.scalar.dma_start(bias_sb[:], bv_r)
            mkrow_sb = consts.tile([1, N], F32R)
            nc.scalar.dma_start(mkrow_sb[:], mk.ap()[None, :])
            ones_sb = consts.tile([1, P], F32)
            nc.vector.memset(ones_sb[:], 1.0)
            mask_sb = consts.tile([P, N], F32)
            for n in range(NN):
                # One full rotation of the shared 8-bank psum tag; the DVE
                # copies release the slots before the first real group lands.
                mps = psp.tile([P, NT], F32, name=f"mps_{n}", tag="ps")
                nc.tensor.matmul(mps[:], ones_sb[:].bitcast(F32R),
                                 mkrow_sb[:, n * NT:(n + 1) * NT],
                                 start=True, stop=True)
                nc.vector.tensor_copy(mask_sb[:, n * NT:(n + 1) * NT], mps[:])
            # Weights in per-(o-half, c) 256 KB chunks: the first matmul is
            # gated by just w[og0,c0]+x[c0] (~0.75 MB), and the og0 pass only
            # needs half the weight bytes up front.
            OH = O // 2
            w_sb = consts.tile([P, CO, O], F32R)
            for og in range(2):
                for c in range(CO):
                    nc.scalar.dma_start(w_sb[:, c, og * OH:(og + 1) * OH],
                                        wt_r[:, c, og * OH:(og + 1) * OH])

            for ns in range(NSUPS):
                x_sb = xs.tile([P, CO, NSUP], F32R)
                for c in range(CO):
                    nc.sync.dma_start(
                        x_sb[:, c, :], x_r[:, c, ns * NSUP:(ns + 1) * NSUP])
                for og in range(2):
                    # 8 psum groups = 4 o-chunks x 2 n-subchunks; each weight
                    # tile feeds 2 back-to-back matmuls (nsub pair).
                    pss = [[psp.tile([P, NT], F32, name=f"ps_{ns}_{og}_{o4}_{nsub}",
                                     tag="ps")
                            for nsub in range(NSUB)] for o4 in range(4)]
                    for c in range(CO):
                        for o4 in range(4):
                            o = og * 4 + o4
                            for nsub in range(NSUB):
                                nc.tensor.matmul(
                                    pss[o4][nsub][:],
                                    w_sb[:, c, o * P:(o + 1) * P],
                                    x_sb[:, c, nsub * NT:(nsub + 1) * NT],
                                    start=(c == 0),
                                    stop=(c == CO - 1),
                                )
                    for o4 in range(4):
                        o = og * 4 + o4
                        for nsub in range(NSUB):
                            n0 = ns * NSUP + nsub * NT
                            ot = outs.tile([P, NT], F32,
                                           name=f"ot_{ns}_{og}_{o4}_{nsub}",
                                           tag="ot")
                            nc.vector.scalar_tensor_tensor(
                                ot[:], pss[o4][nsub][:],
                                bias_sb[:, o:o + 1], mask_sb[:, n0:n0 + NT],
                                op0=mybir.AluOpType.add, op1=mybir.AluOpType.mult,
                            )
                            nc.scalar.dma_start(
                                out.ap()[o * P:(o + 1) * P, n0:n0 + NT], ot[:])
    nc.compile()
    return nc


_NC_CACHE = None


def _get_module():
    global _NC_CACHE
    if _NC_CACHE is None:
        _NC_CACHE = build_module()
    return _NC_CACHE


def _make_in_maps(cond, x_mask, W, b):
    wt = np.ascontiguousarray(W.sum(axis=0).T, dtype=np.float32)   # [C, O]
    bv = np.ascontiguousarray(b.sum(axis=0), dtype=np.float32)     # [O]
    in_maps = []
    for core in range(N_CORES):
        in_maps.append({
            "x": np.ascontiguousarray(cond[core], dtype=np.float32),
            "wt": wt,
            "bv": bv,
            "mk": np.ascontiguousarray(x_mask[core, 0], dtype=np.float32),
        })
    return in_maps


def run(cond, x_mask, W, b, trace=False, trace_cores=None):
    """Run on hardware; returns (out [B,O,N] fp32, BassKernelResults)."""
    nc = _get_module()
    in_maps = _make_in_maps(cond, x_mask, W, b)
    res = run_bass_kernel_spmd(
        nc, in_maps, core_ids=list(range(N_CORES)),
        trace=trace, trace_cores=trace_cores,
    )
    out = np.stack([res.results[i]["out"] for i in range(N_CORES)], axis=0)
    return out, res


def kernel(cond, x_mask, W, b):
    out, _ = run(cond, x_mask, W, b)
    return out
